# revision 49
# baseline (speedup 1.0000x reference)
"""Per-column activation-select kernel for Trainium2 (8 NeuronCores, SPMD).

Problem: out[b, n] = act_{codes[n]}(x[b, n]) with 6 activations
(relu, sigmoid, tanh, elu, leaky_relu(0.01), gelu-tanh-approx),
x: [64, 128, 56, 56] f32, codes: [401408] int32.

Strategy v2 (vs v1's fp16+int8 two-plane layout):
  - Shard batch (64) across 8 cores -> 8 rows/core.
  - Columns permuted host-side into 6 code-pure segments (elu, relu,
    leaky, gelu, tanh, sigmoid), each padded to a multiple of 4 columns
    of 128 partitions (alignment for DVE 2x modes).  One activation per
    contiguous column range on device; inverse permutation host-side.
  - Transport is int8 both ways for 5 of 6 segments (memory-regime
    problem); sigmoid returns f16 tanh(x/2) since the engines, not the
    DMA, are the bottleneck (measured same-run: DMA-only 12.9us at
    ~500 GB/s/core, compute-only 15.5us) -- killing one DVE requant pass
    for a small extra f16 stream on the underused DMA.  Moving tanh to
    f16 as well tips the DMA into the bottleneck (7.5 MB ~15.1us), so
    it stays int8.  Per-segment affine codes:
      elu/relu/leaky/gelu: s = amax_seg/127 shared in/out (relu & leaky
        become exact integer maps); tanh: in clip 4.25, out 1/127;
      sigmoid: in clip 6.0, out f16 t = tanh(x/2), host 0.5*t+0.5.
    int8 writes are round-to-nearest-even + saturating (probed on HW).
    Measured rel err 1.06e-2 < 2e-2 tolerance.
  - One ACT table set (exp_and_others) -> no table switching.  GPSIMD is
    unusable here (neuronxcc rejects TensorScalarPtr and int8 arithmetic
    on Pool), so work balances across ACT and DVE only:
      elu      ACT e = exp(s*q);   DVE q <- relu(q) + (1/s)(min(e,1)-1)
      gelu     ACT t = tanh(0.8727*s*q); DVE q <- q*(0.5 + 0.50198*t)
               (3-param fit of tanh-approx gelu, sup err 0.012)
      tanh     ACT t = tanh(s*q);  DVE q <- 127*t  (tensor_scalar 2x)
      sigmoid  ACT f16 out = tanh(.5*s*q), no requant
      relu     DVE q <- max(q, 0)  (1-src tensor_scalar -> 2x mode)
      leaky    split: 56% DVE STT max(.01q, q) / 44% ACT Prelu(.01)
    Engine budget/rep: ACT ~15.5us = DVE ~15.5us vs DMA 7.0 MB ~14.0us.
  - DRAM layout [P, RPC, F] per core -> one contiguous descriptor per
    partition per block DMA (nr*F bytes), minimal descriptor overhead.
"""
import sys

import numpy as np

sys.path.insert(0, "/opt/trn_rl_repo")

B, C, H, W = 64, 128, 56, 56
N = C * H * W            # 401408
P = 128                  # SBUF partitions
NCORES = 8
RPC = B // NCORES        # rows per core
BLOCK = 4                # rows per tile
PREFETCH = 3             # in-DMA issue distance (blocks ahead of compute)
NUM_ACTS = 6
# segment order: elu, relu, leaky, gelu, tanh, sigmoid (code ids)
SEG = (3, 0, 4, 5, 2, 1)
Q_TANH = 4.25            # tanh input clip; 1-tanh(4.25) = 4e-4
Q_SIG = 6.0              # sigmoid input clip; 1-sigmoid(6) = 2.5e-3
S_TANH = Q_TANH / 127.0
S_SIG = Q_SIG / 127.0
GELU_B = 0.87271875      # gelu ~= x*(c1 + c2*tanh(b*x)), sup err 0.0121
GELU_C1 = 0.5
GELU_C2 = 0.501984375
# fixup-op placement (see engine budget in the header):
#   relu  -> DVE tensor_scalar_max ("vector"; "pool_tt" rejected by neuronxcc)
#   leaky -> split: first LEAKY_DVE_FRAC cols on DVE STT, rest ACT Prelu
ENG_RELU = "vector"
OUT_DMA = "sync"         # out-DMA issue queue: "sync" (SP HWDGE) | "gpsimd"
XP_BUFS = 5              # in/out int8 tile ring depth
SP_BUFS = 5              # f16 scratch tile ring depth
# tanh+sigmoid transported out as f16 tanh (no DVE requant passes; DMA has
# headroom: 497 GB/s measured, engines are the bottleneck)
SIG_F16_OUT = True
TANH_F16_OUT = False
LEAKY_DVE_FRAC = (0.83 if TANH_F16_OUT else 0.56) if SIG_F16_OUT else 0.28
# one ACT Tanh pass covering tanh+sigmoid (host gives sigmoid inputs scale
# 2*S_TANH). Measured +9.5us/rep SLOWER on HW than two passes -- keep off.
MERGE_B = False

_cache = {}


def _register_op(name, make_spec):
    if name in _cache:
        return _cache[name]
    import re

    from concourse.dve_ops import OPS, DveOp

    for op in OPS:
        if op.name == name:
            _cache[name] = op
            return op
    op = DveOp(name, make_spec(), subdim=False, uops_sha={})
    OPS.append(op)
    from concourse import dve_ops as _do

    _do._SUB_OPCODE_FOR_NAME[op.name] = _do._CUSTOM_DVE_ROW_BASE + len(OPS) - 1
    assert _do._SUB_OPCODE_FOR_NAME[op.name] < 0x20
    _do.CUSTOM_DVE_SPECS[op.name] = op.spec
    for ver in ("v3", "v4"):
        try:
            op.compile(ver)
        except ValueError as e:
            m = re.search(r'\]="([0-9a-f]+)"', str(e))
            op.uops_sha[ver] = m.group(1)
            op.compile(ver)
    _cache[name] = op
    return op


def _elu_q_op():
    """out = relu(in0) + C0*(min(in1, 1) - 1)  (elu in q-units, C0=1/s)."""
    def mk():
        from concourse.dve_spec import C0, One, Spec, Src0, Src1, minn, relu

        return Spec(
            body=relu(Src0) + C0 * (minn(Src1, One) - One),
            reference=lambda in0, in1, s0, *cs: np.maximum(in0, 0)
            + s0 * (np.minimum(in1.reshape(in0.shape), 1) - 1),
        )

    return _register_op("ELU_Q_ANT", mk)


def _gelu_q_op():
    """out = in0 * (C0 + C1*in1)  (gelu in q-units when in1=tanh(b*x))."""
    def mk():
        from concourse.dve_spec import C0, C1, Spec, Src0, Src1

        return Spec(
            body=Src0 * (C0 + C1 * Src1),
            reference=lambda in0, in1, s0, s1, *cs: in0
            * (s0 + s1 * in1.reshape(in0.shape)),
        )

    return _register_op("GELU_Q_ANT", mk)


def _build_module(widths: tuple, scales: tuple, reps: int = 1,
                  mode: str = "full"):
    """widths: 6 segment widths (cols); scales: (s_elu, s_relu, s_leaky, s_gelu).
    mode: "full" | "dma" (transfers only) | "compute" (engines only)."""
    import concourse.bacc as bacc
    import concourse.mybir as mybir
    from concourse import tile

    AF = mybir.ActivationFunctionType
    ALU = mybir.AluOpType
    F16 = mybir.dt.float16
    I8 = mybir.dt.int8

    F = int(sum(widths))
    edges = np.concatenate([[0], np.cumsum(widths)]).astype(int)
    W_ELU, W_RELU, W_LEAKY, W_GELU, W_TANH, W_SIG = (int(w) for w in widths)
    s_elu, s_relu, s_leaky, s_gelu = (float(s) for s in scales)

    NF16 = (W_TANH if TANH_F16_OUT else 0) + (W_SIG if SIG_F16_OUT else 0)
    F5 = F - NF16

    nc = bacc.Bacc(target_bir_lowering=False, debug=False)
    xq_in = nc.dram_tensor("xq", [P, RPC, F], I8, kind="ExternalInput").ap()
    outq = nc.dram_tensor("outq", [P, RPC, F5], I8, kind="ExternalOutput").ap()
    if NF16:
        outs = nc.dram_tensor("outs", [P, RPC, NF16], F16,
                              kind="ExternalOutput").ap()

    # leaky split point (cols on DVE; rest on ACT Prelu)
    wd_leaky = int(round(W_LEAKY * LEAKY_DVE_FRAC / 4.0)) * 4
    wd_leaky = max(0, min(W_LEAKY, wd_leaky))

    with tile.TileContext(nc) as tc:
        with (
            tc.tile_pool(name="xp", bufs=XP_BUFS) as xpool,
            tc.tile_pool(name="sp", bufs=SP_BUFS) as spool,
            tc.tile_pool(name="zp", bufs=1) as zpool,
        ):
            nblocks = (RPC + BLOCK - 1) // BLOCK

            def sl(t, i):
                return t[:, :, int(edges[i]):int(edges[i + 1])]

            zeros = None
            if ENG_RELU == "pool_tt":
                zeros = zpool.tile([P, BLOCK, W_RELU], I8, tag="z", name="zeros")
                nc.vector.memset(zeros[:], 0)

            tiles = {}
            total = reps * nblocks

            def fetch(k):
                rep, nb = k // nblocks, k % nblocks
                r0 = nb * BLOCK
                nr = min(BLOCK, RPC - r0)
                tq = xpool.tile([P, nr, F], I8, tag="xq", name=f"xq{rep}_{nb}")
                if mode != "compute":
                    nc.sync.dma_start(tq[:], xq_in[:, r0:r0 + nr])
                tiles[k] = tq

            for k in range(min(PREFETCH, total)):
                fetch(k)
            for kk in range(total):
                rep, nb = kk // nblocks, kk % nblocks
                if kk + PREFETCH < total:
                    fetch(kk + PREFETCH)
                tq = tiles.pop(kk)
                r0 = nb * BLOCK
                nr = min(BLOCK, RPC - r0)

                if mode == "dma":
                    nc.sync.dma_start(outq[:, r0:r0 + nr], tq[:, :, :F5])
                    continue

                lk = sl(tq, 2)  # leaky segment
                lk_d = lk[:, :, :wd_leaky]
                lk_a = lk[:, :, wd_leaky:]

                # --- DVE: leaky (no ACT dep) first, then chained fixups ---
                if wd_leaky:
                    nc.vector.scalar_tensor_tensor(
                        lk_d, lk_d, 0.01, lk_d, op0=ALU.mult, op1=ALU.max,
                    )

                # --- ACT table passes (all in exp_and_others) ---
                e = spool.tile([P, nr, W_ELU], F16, tag="e", name=f"e{rep}_{nb}")
                nc.scalar.activation(e[:], sl(tq, 0), AF.Exp, scale=s_elu)
                tg = spool.tile([P, nr, W_GELU], F16, tag="tg", name=f"tg{rep}_{nb}")
                nc.scalar.activation(tg[:], sl(tq, 3), AF.Tanh,
                                     scale=GELU_B * s_gelu)
                if MERGE_B or (TANH_F16_OUT and SIG_F16_OUT):
                    # adjacent tanh+sig segments share one f16 tile; with
                    # MERGE_B the host made both input scales equal so ONE
                    # Tanh pass covers both segments
                    tb = spool.tile([P, nr, W_TANH + W_SIG], F16, tag="tb",
                                    name=f"tb{rep}_{nb}")
                    tt = tb[:, :, :W_TANH]
                    ts = tb[:, :, W_TANH:]
                else:
                    tb = None
                    tt = spool.tile([P, nr, W_TANH], F16, tag="tt",
                                    name=f"tt{rep}_{nb}")[:]
                    ts = spool.tile([P, nr, W_SIG], F16, tag="ts",
                                    name=f"ts{rep}_{nb}")[:]
                if MERGE_B:
                    nc.scalar.activation(
                        tb[:, :, :], tq[:, :, int(edges[4]):int(edges[6])],
                        AF.Tanh, scale=S_TANH)
                else:
                    nc.scalar.activation(tt, sl(tq, 4), AF.Tanh, scale=S_TANH)
                    nc.scalar.activation(ts, sl(tq, 5), AF.Tanh,
                                         scale=0.5 * S_SIG)
                if wd_leaky < W_LEAKY:
                    nc.scalar.activation(lk_a, lk_a, AF.Prelu, alpha=0.01)

                # --- relu on Pool (tensor_tensor max with zeros) ---
                if ENG_RELU == "pool_tt":
                    nc.gpsimd.tensor_tensor(
                        sl(tq, 1), sl(tq, 1), zeros[:, :nr], op=ALU.max
                    )
                else:
                    nc.vector.tensor_scalar_max(sl(tq, 1), sl(tq, 1), 0.0)

                # --- remaining DVE fixups ---
                nc.vector._custom_dve(
                    _elu_q_op(), out=sl(tq, 0), in0=sl(tq, 0), in1=e[:],
                    s0=1.0 / s_elu,
                )
                nc.vector._custom_dve(
                    _gelu_q_op(), out=sl(tq, 3), in0=sl(tq, 3), in1=tg[:],
                    s0=GELU_C1, s1=GELU_C2,
                )
                # int8 writes round-to-nearest-even + saturate (probed on HW)
                if not TANH_F16_OUT:
                    nc.vector.tensor_scalar(
                        sl(tq, 4), tt, 127.0, 0.0, op0=ALU.mult, op1=ALU.add
                    )
                if not SIG_F16_OUT:
                    nc.vector.tensor_scalar(
                        sl(tq, 5), ts, 63.5, 64.0, op0=ALU.mult, op1=ALU.add
                    )

                if mode != "compute":
                    oeng = nc.gpsimd if OUT_DMA == "gpsimd" else nc.sync
                    if TANH_F16_OUT and SIG_F16_OUT:
                        oeng.dma_start(outs[:, r0:r0 + nr], tb[:])
                    elif SIG_F16_OUT:
                        oeng.dma_start(outs[:, r0:r0 + nr], ts)
                    oeng.dma_start(outq[:, r0:r0 + nr], tq[:, :, :F5])

    nc.compile()
    return nc


def _get_module(widths: tuple, scales: tuple, reps: int = 1,
                mode: str = "full"):
    key = ("nc", widths, scales, reps, BLOCK, PREFETCH, XP_BUFS, SP_BUFS,
           ENG_RELU, OUT_DMA, LEAKY_DVE_FRAC, SIG_F16_OUT, TANH_F16_OUT,
           MERGE_B, mode)
    if key not in _cache:
        _cache[key] = _build_module(widths, scales, reps, mode)
    return _cache[key]


def _plan(codes: np.ndarray):
    """Single-plane column permutation plan.

    widths  : per-segment padded widths (cols of 128), multiple of 4
    inv     : source flat column for each padded [p, f] slot
              (padding slots replicate the segment's first column)
    cols    : original column ids in segment order (unpadded)
    fl      : padded [p, f] flat slot holding each cols entry
    """
    key = codes.tobytes()
    if ("plan", key) in _cache:
        return _cache[("plan", key)]
    codes = codes.astype(np.int64)
    assert codes.shape == (N,) and codes.min() >= 0 and codes.max() < NUM_ACTS

    rank = np.full(NUM_ACTS, -1, np.int64)
    for i, k in enumerate(SEG):
        rank[k] = i
    seg = rank[codes]
    cols_sorted = np.argsort(seg, kind="stable")
    counts = np.bincount(seg, minlength=len(SEG))[:len(SEG)]
    # ceil(c/P) rounded up to a multiple of 4 columns
    widths = tuple(int(((-(-c // P)) + 3) // 4 * 4) for c in counts)
    col_base = np.concatenate([[0], np.cumsum(widths)])
    F = int(col_base[-1])
    elem_base = np.repeat(col_base[:len(SEG)] * P, counts)
    cnt_base = np.concatenate([[0], np.cumsum(counts)])
    within = np.arange(N) - np.repeat(cnt_base[:len(SEG)], counts)
    q = elem_base + within
    fl = (q % P) * F + q // P
    inv = np.empty(P * F, np.int64)
    inv2 = inv.reshape(P, F)
    # padding slots replicate each segment's first column (same code)
    for i in range(len(SEG)):
        first = cols_sorted[cnt_base[i]] if counts[i] else 0
        inv2[:, int(col_base[i]):int(col_base[i + 1])] = first
    inv[fl] = cols_sorted
    plan = (widths, inv.astype(np.int64), cols_sorted.astype(np.int64),
            fl.astype(np.int64), counts)
    _cache[("plan", key)] = plan
    return plan


def _prep_inputs(x: np.ndarray, codes: np.ndarray):
    """Permuted per-core int8 inputs [NCORES, P, RPC, F] + decode vectors."""
    widths, inv, cols, fl, counts = _plan(codes)
    F = int(sum(widths))
    col_base = np.concatenate([[0], np.cumsum(widths)]).astype(int)
    x2 = np.asarray(x, dtype=np.float32).reshape(B, N)

    # per-segment input scales (A segments data-dependent, tanh/sig fixed)
    amax = np.empty(len(SEG), np.float32)
    cnt_base = np.concatenate([[0], np.cumsum(counts)])
    for i in range(len(SEG)):
        cs = cols[cnt_base[i]:cnt_base[i + 1]]
        amax[i] = max(float(np.abs(x2[:, cs]).max()), 1e-6) if len(cs) else 1.0
    s_elu, s_relu, s_leaky, s_gelu = (float(amax[i] / 127.0) for i in range(4))
    s_sig_in = 2.0 * S_TANH if MERGE_B else S_SIG
    seg_in_scale = np.array([s_elu, s_relu, s_leaky, s_gelu, S_TANH, s_sig_in],
                            np.float32)
    # decode: y = a*q + b per column
    seg_a = np.array([s_elu, s_relu, s_leaky, s_gelu, 1.0 / 127.0, 1.0 / 127.0],
                     np.float32)
    seg_b = np.array([0.0, 0.0, 0.0, 0.0, 0.0, 0.5 - 64.0 / 127.0],
                     np.float32)
    a_col = np.empty(F, np.float32)
    b_col = np.empty(F, np.float32)
    sc_col = np.empty(F, np.float32)
    for i in range(len(SEG)):
        a_col[col_base[i]:col_base[i + 1]] = seg_a[i]
        b_col[col_base[i]:col_base[i + 1]] = seg_b[i]
        sc_col[col_base[i]:col_base[i + 1]] = seg_in_scale[i]

    xpf = x2[:, inv]                                   # [B, P*F] f32
    xq = np.clip(np.rint(xpf.reshape(B, P, F) / sc_col[None, None, :]),
                 -127, 127).astype(np.int8)
    # [B, P, F] -> [NCORES, P, RPC, F]
    xq = xq.reshape(NCORES, RPC, P, F).transpose(0, 2, 1, 3).copy()
    scales = (round(s_elu, 8), round(s_relu, 8), round(s_leaky, 8),
              round(s_gelu, 8))
    return widths, scales, xq, (a_col, b_col), (cols, fl)


def kernel(x: np.ndarray, act_codes: np.ndarray) -> np.ndarray:
    from concourse.bass_utils import run_bass_kernel_spmd

    codes = np.asarray(act_codes, dtype=np.int32)
    widths, scales, xq, (a_col, b_col), (cols, fl) = _prep_inputs(x, codes)
    F = int(sum(widths))
    nc = _get_module(widths, scales)

    in_maps = [{"xq": xq[c]} for c in range(NCORES)]
    res = run_bass_kernel_spmd(nc, in_maps, list(range(NCORES)))

    W_TANH, W_SIG = int(widths[4]), int(widths[5])
    NF16 = (W_TANH if TANH_F16_OUT else 0) + (W_SIG if SIG_F16_OUT else 0)
    F5 = F - NF16
    y = np.empty((B, P, F), dtype=np.float32)
    outq = np.empty((B, P, F5), dtype=np.int8)
    for c in range(NCORES):
        # device out [P, RPC, F5] -> rows [RPC, P, F5]
        outq[c * RPC:(c + 1) * RPC] = res.results[c]["outq"].transpose(1, 0, 2)
    y[:, :, :F5] = (outq.astype(np.float32) * a_col[None, None, :F5]
                    + b_col[None, None, :F5])
    if NF16:
        t = np.empty((B, P, NF16), dtype=np.float16)
        for c in range(NCORES):
            t[c * RPC:(c + 1) * RPC] = res.results[c]["outs"].transpose(1, 0, 2)
        tf = t.astype(np.float32)
        off = 0
        if TANH_F16_OUT:
            y[:, :, F5:F5 + W_TANH] = tf[:, :, :W_TANH]  # tanh directly
            off = W_TANH
        if SIG_F16_OUT:
            # sigmoid = 0.5*tanh(x/2) + 0.5
            y[:, :, F5 + off:] = tf[:, :, off:] * 0.5 + 0.5
    y = y.reshape(B, P * F)
    out2 = np.empty((B, N), dtype=np.float32)
    out2[:, cols] = y[:, fl]
    return out2.reshape(B, C, H, W)


# revision 50
# speedup vs baseline: 1.1675x; 1.1675x over previous
"""Per-column activation-select kernel for Trainium2 (8 NeuronCores, SPMD).

Problem: out[b, n] = act_{codes[n]}(x[b, n]) with 6 activations
(relu, sigmoid, tanh, elu, leaky_relu(0.01), gelu-tanh-approx),
x: [64, 128, 56, 56] f32, codes: [401408] int32.

Strategy v2 (vs v1's fp16+int8 two-plane layout):
  - Shard batch (64) across 8 cores -> 8 rows/core.
  - Columns permuted host-side into 6 code-pure segments (elu, relu,
    leaky, gelu, tanh, sigmoid), each padded to a multiple of 4 columns
    of 128 partitions (alignment for DVE 2x modes).  One activation per
    contiguous column range on device; inverse permutation host-side.
  - Transport is int8 both ways for 5 of 6 segments (memory-regime
    problem); sigmoid returns f16 tanh(x/2) since the engines, not the
    DMA, are the bottleneck (measured same-run: DMA-only 12.9us at
    ~500 GB/s/core, compute-only 15.5us) -- killing one DVE requant pass
    for a small extra f16 stream on the underused DMA.  Moving tanh to
    f16 as well tips the DMA into the bottleneck (7.5 MB ~15.1us), so
    it stays int8.  Per-segment affine codes:
      elu/relu/leaky/gelu: s = amax_seg/127 shared in/out (relu & leaky
        become exact integer maps); tanh: in clip 4.25, out 1/127;
      sigmoid: in clip 6.0, out f16 t = tanh(x/2), host 0.5*t+0.5.
    int8 writes are round-to-nearest-even + saturating (probed on HW).
    Measured rel err 1.06e-2 < 2e-2 tolerance.
  - One ACT table set (exp_and_others) -> no table switching.  GPSIMD is
    unusable here (neuronxcc rejects TensorScalarPtr and int8 arithmetic
    on Pool), so work balances across ACT and DVE only:
      elu      ACT e = exp(s*q);   DVE q <- relu(q) + (1/s)(min(e,1)-1)
      gelu     ACT t = tanh(0.8727*s*q); DVE q <- q*(0.5 + 0.50198*t)
               (3-param fit of tanh-approx gelu, sup err 0.012)
      tanh     ACT t = tanh(s*q);  DVE q <- 127*t  (tensor_scalar 2x)
      sigmoid  ACT f16 out = tanh(.5*s*q), no requant
      relu     DVE q <- max(q, 0)  (1-src tensor_scalar -> 2x mode)
      leaky    split: 56% DVE STT max(.01q, q) / 44% ACT Prelu(.01)
    Engine budget/rep: ACT ~15.5us = DVE ~15.5us vs DMA 7.0 MB ~14.0us.
  - DRAM layout [P, RPC, F] per core -> one contiguous descriptor per
    partition per block DMA (nr*F bytes), minimal descriptor overhead.
"""
import sys

import numpy as np

sys.path.insert(0, "/opt/trn_rl_repo")

B, C, H, W = 64, 128, 56, 56
N = C * H * W            # 401408
P = 128                  # SBUF partitions
NCORES = 8
RPC = B // NCORES        # rows per core
BLOCK = 8                # rows per tile
PREFETCH = 3             # in-DMA issue distance (blocks ahead of compute)
NUM_ACTS = 6
# segment order: elu, relu, leaky, gelu, tanh, sigmoid (code ids)
SEG = (3, 0, 4, 5, 2, 1)
Q_TANH = 4.25            # tanh input clip; 1-tanh(4.25) = 4e-4
Q_SIG = 6.0              # sigmoid input clip; 1-sigmoid(6) = 2.5e-3
S_TANH = Q_TANH / 127.0
S_SIG = Q_SIG / 127.0
GELU_B = 0.87271875      # gelu ~= x*(c1 + c2*tanh(b*x)), sup err 0.0121
GELU_C1 = 0.5
GELU_C2 = 0.501984375
# fixup-op placement (see engine budget in the header):
#   relu  -> DVE tensor_scalar_max ("vector"; "pool_tt" rejected by neuronxcc)
#   leaky -> split: first LEAKY_DVE_FRAC cols on DVE STT, rest ACT Prelu
ENG_RELU = "vector"
OUT_DMA = "sync"         # out-DMA issue queue: "sync" (SP HWDGE) | "gpsimd"
XP_BUFS = 5              # in/out int8 tile ring depth (5x25KB/partition)
SP_BUFS = 2              # f16 scratch tile ring depth
# tanh+sigmoid transported out as f16 tanh (no DVE requant passes; DMA has
# headroom: 497 GB/s measured, engines are the bottleneck)
SIG_F16_OUT = True
TANH_F16_OUT = False
LEAKY_DVE_FRAC = (0.83 if TANH_F16_OUT else 0.56) if SIG_F16_OUT else 0.28
# one ACT Tanh pass covering tanh+sigmoid (host gives sigmoid inputs scale
# 2*S_TANH). Measured +9.5us/rep SLOWER on HW than two passes -- keep off.
MERGE_B = False

_cache = {}


def _register_op(name, make_spec):
    if name in _cache:
        return _cache[name]
    import re

    from concourse.dve_ops import OPS, DveOp

    for op in OPS:
        if op.name == name:
            _cache[name] = op
            return op
    op = DveOp(name, make_spec(), subdim=False, uops_sha={})
    OPS.append(op)
    from concourse import dve_ops as _do

    _do._SUB_OPCODE_FOR_NAME[op.name] = _do._CUSTOM_DVE_ROW_BASE + len(OPS) - 1
    assert _do._SUB_OPCODE_FOR_NAME[op.name] < 0x20
    _do.CUSTOM_DVE_SPECS[op.name] = op.spec
    for ver in ("v3", "v4"):
        try:
            op.compile(ver)
        except ValueError as e:
            m = re.search(r'\]="([0-9a-f]+)"', str(e))
            op.uops_sha[ver] = m.group(1)
            op.compile(ver)
    _cache[name] = op
    return op


def _elu_q_op():
    """out = relu(in0) + C0*(min(in1, 1) - 1)  (elu in q-units, C0=1/s)."""
    def mk():
        from concourse.dve_spec import C0, One, Spec, Src0, Src1, minn, relu

        return Spec(
            body=relu(Src0) + C0 * (minn(Src1, One) - One),
            reference=lambda in0, in1, s0, *cs: np.maximum(in0, 0)
            + s0 * (np.minimum(in1.reshape(in0.shape), 1) - 1),
        )

    return _register_op("ELU_Q_ANT", mk)


def _gelu_q_op():
    """out = in0 * (C0 + C1*in1)  (gelu in q-units when in1=tanh(b*x))."""
    def mk():
        from concourse.dve_spec import C0, C1, Spec, Src0, Src1

        return Spec(
            body=Src0 * (C0 + C1 * Src1),
            reference=lambda in0, in1, s0, s1, *cs: in0
            * (s0 + s1 * in1.reshape(in0.shape)),
        )

    return _register_op("GELU_Q_ANT", mk)


def _build_module(widths: tuple, scales: tuple, reps: int = 1,
                  mode: str = "full"):
    """widths: 6 segment widths (cols); scales: (s_elu, s_relu, s_leaky, s_gelu).
    mode: "full" | "dma" (transfers only) | "compute" (engines only)."""
    import concourse.bacc as bacc
    import concourse.mybir as mybir
    from concourse import tile

    AF = mybir.ActivationFunctionType
    ALU = mybir.AluOpType
    F16 = mybir.dt.float16
    I8 = mybir.dt.int8

    F = int(sum(widths))
    edges = np.concatenate([[0], np.cumsum(widths)]).astype(int)
    W_ELU, W_RELU, W_LEAKY, W_GELU, W_TANH, W_SIG = (int(w) for w in widths)
    s_elu, s_relu, s_leaky, s_gelu = (float(s) for s in scales)

    NF16 = (W_TANH if TANH_F16_OUT else 0) + (W_SIG if SIG_F16_OUT else 0)
    F5 = F - NF16

    nc = bacc.Bacc(target_bir_lowering=False, debug=False)
    xq_in = nc.dram_tensor("xq", [P, RPC, F], I8, kind="ExternalInput").ap()
    outq = nc.dram_tensor("outq", [P, RPC, F5], I8, kind="ExternalOutput").ap()
    if NF16:
        outs = nc.dram_tensor("outs", [P, RPC, NF16], F16,
                              kind="ExternalOutput").ap()

    # leaky split point (cols on DVE; rest on ACT Prelu)
    wd_leaky = int(round(W_LEAKY * LEAKY_DVE_FRAC / 4.0)) * 4
    wd_leaky = max(0, min(W_LEAKY, wd_leaky))

    with tile.TileContext(nc) as tc:
        with (
            tc.tile_pool(name="xp", bufs=XP_BUFS) as xpool,
            tc.tile_pool(name="sp", bufs=SP_BUFS) as spool,
            tc.tile_pool(name="zp", bufs=1) as zpool,
        ):
            nblocks = (RPC + BLOCK - 1) // BLOCK

            def sl(t, i):
                return t[:, :, int(edges[i]):int(edges[i + 1])]

            zeros = None
            if ENG_RELU == "pool_tt":
                zeros = zpool.tile([P, BLOCK, W_RELU], I8, tag="z", name="zeros")
                nc.vector.memset(zeros[:], 0)

            tiles = {}
            total = reps * nblocks

            def fetch(k):
                rep, nb = k // nblocks, k % nblocks
                r0 = nb * BLOCK
                nr = min(BLOCK, RPC - r0)
                tq = xpool.tile([P, nr, F], I8, tag="xq", name=f"xq{rep}_{nb}")
                if mode != "compute":
                    nc.sync.dma_start(tq[:], xq_in[:, r0:r0 + nr])
                tiles[k] = tq

            for k in range(min(PREFETCH, total)):
                fetch(k)
            for kk in range(total):
                rep, nb = kk // nblocks, kk % nblocks
                if kk + PREFETCH < total:
                    fetch(kk + PREFETCH)
                tq = tiles.pop(kk)
                r0 = nb * BLOCK
                nr = min(BLOCK, RPC - r0)

                if mode == "dma":
                    nc.sync.dma_start(outq[:, r0:r0 + nr], tq[:, :, :F5])
                    continue

                lk = sl(tq, 2)  # leaky segment
                lk_d = lk[:, :, :wd_leaky]
                lk_a = lk[:, :, wd_leaky:]

                # --- DVE: leaky (no ACT dep) first, then chained fixups ---
                if wd_leaky:
                    nc.vector.scalar_tensor_tensor(
                        lk_d, lk_d, 0.01, lk_d, op0=ALU.mult, op1=ALU.max,
                    )

                # --- ACT table passes (all in exp_and_others) ---
                e = spool.tile([P, nr, W_ELU], F16, tag="e", name=f"e{rep}_{nb}")
                nc.scalar.activation(e[:], sl(tq, 0), AF.Exp, scale=s_elu)
                tg = spool.tile([P, nr, W_GELU], F16, tag="tg", name=f"tg{rep}_{nb}")
                nc.scalar.activation(tg[:], sl(tq, 3), AF.Tanh,
                                     scale=GELU_B * s_gelu)
                if MERGE_B or (TANH_F16_OUT and SIG_F16_OUT):
                    # adjacent tanh+sig segments share one f16 tile; with
                    # MERGE_B the host made both input scales equal so ONE
                    # Tanh pass covers both segments
                    tb = spool.tile([P, nr, W_TANH + W_SIG], F16, tag="tb",
                                    name=f"tb{rep}_{nb}")
                    tt = tb[:, :, :W_TANH]
                    ts = tb[:, :, W_TANH:]
                else:
                    tb = None
                    tt = spool.tile([P, nr, W_TANH], F16, tag="tt",
                                    name=f"tt{rep}_{nb}")[:]
                    ts = spool.tile([P, nr, W_SIG], F16, tag="ts",
                                    name=f"ts{rep}_{nb}")[:]
                if MERGE_B:
                    nc.scalar.activation(
                        tb[:, :, :], tq[:, :, int(edges[4]):int(edges[6])],
                        AF.Tanh, scale=S_TANH)
                else:
                    nc.scalar.activation(tt, sl(tq, 4), AF.Tanh, scale=S_TANH)
                    nc.scalar.activation(ts, sl(tq, 5), AF.Tanh,
                                         scale=0.5 * S_SIG)
                if wd_leaky < W_LEAKY:
                    nc.scalar.activation(lk_a, lk_a, AF.Prelu, alpha=0.01)

                # --- relu on Pool (tensor_tensor max with zeros) ---
                if ENG_RELU == "pool_tt":
                    nc.gpsimd.tensor_tensor(
                        sl(tq, 1), sl(tq, 1), zeros[:, :nr], op=ALU.max
                    )
                else:
                    nc.vector.tensor_scalar_max(sl(tq, 1), sl(tq, 1), 0.0)

                # --- remaining DVE fixups ---
                nc.vector._custom_dve(
                    _elu_q_op(), out=sl(tq, 0), in0=sl(tq, 0), in1=e[:],
                    s0=1.0 / s_elu,
                )
                nc.vector._custom_dve(
                    _gelu_q_op(), out=sl(tq, 3), in0=sl(tq, 3), in1=tg[:],
                    s0=GELU_C1, s1=GELU_C2,
                )
                # int8 writes round-to-nearest-even + saturate (probed on HW)
                if not TANH_F16_OUT:
                    nc.vector.tensor_scalar(
                        sl(tq, 4), tt, 127.0, 0.0, op0=ALU.mult, op1=ALU.add
                    )
                if not SIG_F16_OUT:
                    nc.vector.tensor_scalar(
                        sl(tq, 5), ts, 63.5, 64.0, op0=ALU.mult, op1=ALU.add
                    )

                if mode != "compute":
                    oeng = nc.gpsimd if OUT_DMA == "gpsimd" else nc.sync
                    if TANH_F16_OUT and SIG_F16_OUT:
                        oeng.dma_start(outs[:, r0:r0 + nr], tb[:])
                    elif SIG_F16_OUT:
                        oeng.dma_start(outs[:, r0:r0 + nr], ts)
                    oeng.dma_start(outq[:, r0:r0 + nr], tq[:, :, :F5])

    nc.compile()
    return nc


def _get_module(widths: tuple, scales: tuple, reps: int = 1,
                mode: str = "full"):
    key = ("nc", widths, scales, reps, BLOCK, PREFETCH, XP_BUFS, SP_BUFS,
           ENG_RELU, OUT_DMA, LEAKY_DVE_FRAC, SIG_F16_OUT, TANH_F16_OUT,
           MERGE_B, mode)
    if key not in _cache:
        _cache[key] = _build_module(widths, scales, reps, mode)
    return _cache[key]


def _plan(codes: np.ndarray):
    """Single-plane column permutation plan.

    widths  : per-segment padded widths (cols of 128), multiple of 4
    inv     : source flat column for each padded [p, f] slot
              (padding slots replicate the segment's first column)
    cols    : original column ids in segment order (unpadded)
    fl      : padded [p, f] flat slot holding each cols entry
    """
    key = codes.tobytes()
    if ("plan", key) in _cache:
        return _cache[("plan", key)]
    codes = codes.astype(np.int64)
    assert codes.shape == (N,) and codes.min() >= 0 and codes.max() < NUM_ACTS

    rank = np.full(NUM_ACTS, -1, np.int64)
    for i, k in enumerate(SEG):
        rank[k] = i
    seg = rank[codes]
    cols_sorted = np.argsort(seg, kind="stable")
    counts = np.bincount(seg, minlength=len(SEG))[:len(SEG)]
    # ceil(c/P) rounded up to a multiple of 4 columns
    widths = tuple(int(((-(-c // P)) + 3) // 4 * 4) for c in counts)
    col_base = np.concatenate([[0], np.cumsum(widths)])
    F = int(col_base[-1])
    elem_base = np.repeat(col_base[:len(SEG)] * P, counts)
    cnt_base = np.concatenate([[0], np.cumsum(counts)])
    within = np.arange(N) - np.repeat(cnt_base[:len(SEG)], counts)
    q = elem_base + within
    fl = (q % P) * F + q // P
    inv = np.empty(P * F, np.int64)
    inv2 = inv.reshape(P, F)
    # padding slots replicate each segment's first column (same code)
    for i in range(len(SEG)):
        first = cols_sorted[cnt_base[i]] if counts[i] else 0
        inv2[:, int(col_base[i]):int(col_base[i + 1])] = first
    inv[fl] = cols_sorted
    plan = (widths, inv.astype(np.int64), cols_sorted.astype(np.int64),
            fl.astype(np.int64), counts)
    _cache[("plan", key)] = plan
    return plan


def _prep_inputs(x: np.ndarray, codes: np.ndarray):
    """Permuted per-core int8 inputs [NCORES, P, RPC, F] + decode vectors."""
    widths, inv, cols, fl, counts = _plan(codes)
    F = int(sum(widths))
    col_base = np.concatenate([[0], np.cumsum(widths)]).astype(int)
    x2 = np.asarray(x, dtype=np.float32).reshape(B, N)

    # per-segment input scales (A segments data-dependent, tanh/sig fixed)
    amax = np.empty(len(SEG), np.float32)
    cnt_base = np.concatenate([[0], np.cumsum(counts)])
    for i in range(len(SEG)):
        cs = cols[cnt_base[i]:cnt_base[i + 1]]
        amax[i] = max(float(np.abs(x2[:, cs]).max()), 1e-6) if len(cs) else 1.0
    s_elu, s_relu, s_leaky, s_gelu = (float(amax[i] / 127.0) for i in range(4))
    s_sig_in = 2.0 * S_TANH if MERGE_B else S_SIG
    seg_in_scale = np.array([s_elu, s_relu, s_leaky, s_gelu, S_TANH, s_sig_in],
                            np.float32)
    # decode: y = a*q + b per column
    seg_a = np.array([s_elu, s_relu, s_leaky, s_gelu, 1.0 / 127.0, 1.0 / 127.0],
                     np.float32)
    seg_b = np.array([0.0, 0.0, 0.0, 0.0, 0.0, 0.5 - 64.0 / 127.0],
                     np.float32)
    a_col = np.empty(F, np.float32)
    b_col = np.empty(F, np.float32)
    sc_col = np.empty(F, np.float32)
    for i in range(len(SEG)):
        a_col[col_base[i]:col_base[i + 1]] = seg_a[i]
        b_col[col_base[i]:col_base[i + 1]] = seg_b[i]
        sc_col[col_base[i]:col_base[i + 1]] = seg_in_scale[i]

    xpf = x2[:, inv]                                   # [B, P*F] f32
    xq = np.clip(np.rint(xpf.reshape(B, P, F) / sc_col[None, None, :]),
                 -127, 127).astype(np.int8)
    # [B, P, F] -> [NCORES, P, RPC, F]
    xq = xq.reshape(NCORES, RPC, P, F).transpose(0, 2, 1, 3).copy()
    scales = (round(s_elu, 8), round(s_relu, 8), round(s_leaky, 8),
              round(s_gelu, 8))
    return widths, scales, xq, (a_col, b_col), (cols, fl)


def kernel(x: np.ndarray, act_codes: np.ndarray) -> np.ndarray:
    from concourse.bass_utils import run_bass_kernel_spmd

    codes = np.asarray(act_codes, dtype=np.int32)
    widths, scales, xq, (a_col, b_col), (cols, fl) = _prep_inputs(x, codes)
    F = int(sum(widths))
    nc = _get_module(widths, scales)

    in_maps = [{"xq": xq[c]} for c in range(NCORES)]
    res = run_bass_kernel_spmd(nc, in_maps, list(range(NCORES)))

    W_TANH, W_SIG = int(widths[4]), int(widths[5])
    NF16 = (W_TANH if TANH_F16_OUT else 0) + (W_SIG if SIG_F16_OUT else 0)
    F5 = F - NF16
    y = np.empty((B, P, F), dtype=np.float32)
    outq = np.empty((B, P, F5), dtype=np.int8)
    for c in range(NCORES):
        # device out [P, RPC, F5] -> rows [RPC, P, F5]
        outq[c * RPC:(c + 1) * RPC] = res.results[c]["outq"].transpose(1, 0, 2)
    y[:, :, :F5] = (outq.astype(np.float32) * a_col[None, None, :F5]
                    + b_col[None, None, :F5])
    if NF16:
        t = np.empty((B, P, NF16), dtype=np.float16)
        for c in range(NCORES):
            t[c * RPC:(c + 1) * RPC] = res.results[c]["outs"].transpose(1, 0, 2)
        tf = t.astype(np.float32)
        off = 0
        if TANH_F16_OUT:
            y[:, :, F5:F5 + W_TANH] = tf[:, :, :W_TANH]  # tanh directly
            off = W_TANH
        if SIG_F16_OUT:
            # sigmoid = 0.5*tanh(x/2) + 0.5
            y[:, :, F5 + off:] = tf[:, :, off:] * 0.5 + 0.5
    y = y.reshape(B, P * F)
    out2 = np.empty((B, N), dtype=np.float32)
    out2[:, cols] = y[:, fl]
    return out2.reshape(B, C, H, W)
